# revision 5
# baseline (speedup 1.0000x reference)
"""HRR self-attention Trainium2 kernel, v2.

Same math as baseline (see kernel.py docstring): freq-domain qkv GEMM,
complex bind / cumsum / unbind elementwise, irfft+output GEMM — FFTs folded
into the weights host-side.

v2 changes vs baseline:
- DMA priming ordered by first use: (wf_h0[mi], x_sc0[mi]) pairs first so
  the PE's first accumulation chain starts ~0.7us in, then wf_h1 before the
  PE needs cc6+, then x_sc1, wo, x_sc2/3.
- DC/Nyquist fixup lanes are adjacent (lanes 0,1) via a host-side channel
  permutation, so each tile's fixup is ONE 2-partition Pool op instead of
  two 1-partition ops (half the Pool traffic, shorter chain latency).
- Output stored bf16 (halves output DMA); host combines partials in f32.

Sharding: 8 cores = 4 batches x 2 head-groups (4 heads each); host sums the
two head-group partials per batch.

Frequency packing per head (D=128 -> rfft bins 0..64): per pair-of-heads
chunk, re-half lane order is [A-DC, B-DC, A-re1..63, B-re1..63] and im-half
is [A-Nyq, B-Nyq, A-im1..63, B-im1..63]; complex multiply is generic on
lanes 2..127 and real-only on lanes 0..1 (the Pool fixup).
"""

import numpy as np
import ml_dtypes

B, S, M, H = 4, 2048, 1024, 8
D = M // H          # 128
NB = D // 2         # 64 bins per half
SC = 512            # sequence chunk
NSC = S // SC       # 4
NMI = M // 128      # 8 contraction chunks
NCC = 12            # qkv freq channel chunks per core
NCO = 4             # U channel chunks per core

BF16 = ml_dtypes.bfloat16

# channel permutation within a 128-lane half: [A0, B0, A1..A63, B1..B63]
PERM = np.array([0, 64] + list(range(1, 64)) + list(range(65, 128)))


# ---------------------------------------------------------------------------
# Host-side weight fusion
# ---------------------------------------------------------------------------

def _head_blocks(Wh, F):
    """Wh (D, M) spatial head weights -> (re_block, im_block) each (64, M)."""
    FW = F @ Wh  # (65, M) complex
    re = FW.real[0:NB]
    im = np.concatenate([FW.real[NB:NB + 1], FW.imag[1:NB]], axis=0)
    return re, im


def build_tables(W_qkv, W_o):
    """Per-core (WfT [1024,1536] bf16, WoG [512,1024] bf16)."""
    W_qkv = np.asarray(W_qkv, dtype=np.float64)
    W_o = np.asarray(W_o, dtype=np.float64)
    F = np.fft.rfft(np.eye(D), axis=-1).T  # (65, 128)
    Wq = W_qkv[0 * M:1 * M].reshape(H, D, M)
    Wk = W_qkv[1 * M:2 * M].reshape(H, D, M)
    Wv = W_qkv[2 * M:3 * M].reshape(H, D, M)

    # irfft basis in packed-channel order [re 0..63, nyq, im 1..63]
    n = np.arange(D)
    f = np.arange(NB)
    Gr = np.cos(2 * np.pi * np.outer(n, f) / D) / D
    Gr[:, 1:] *= 2.0
    Gnyq = np.cos(np.pi * n)[:, None] / D
    Gi = -2.0 * np.sin(2 * np.pi * np.outer(n, f) / D) / D
    G = np.concatenate([Gr, Gnyq, Gi[:, 1:]], axis=1)  # (128, 128)

    tables = []
    for core in range(8):
        g = core % 2
        heads = [4 * g + i for i in range(4)]
        chunks = []
        out_rows = []
        for pair in range(2):
            hA, hB = heads[2 * pair], heads[2 * pair + 1]
            for Wx in (Wk, Wv, Wq):
                reA, imA = _head_blocks(Wx[hA], F)
                reB, imB = _head_blocks(Wx[hB], F)
                chunks.append(np.concatenate([reA, reB], axis=0)[PERM])
                chunks.append(np.concatenate([imA, imB], axis=0)[PERM])
            WoGA = W_o[:, D * hA:D * (hA + 1)] @ G  # (1024, 128)
            WoGB = W_o[:, D * hB:D * (hB + 1)] @ G
            out_rows.append(
                np.concatenate([WoGA.T[:NB], WoGB.T[:NB]], axis=0)[PERM])
            out_rows.append(
                np.concatenate([WoGA.T[NB:], WoGB.T[NB:]], axis=0)[PERM])
        WfT = np.concatenate(chunks, axis=0).T  # (1024, 1536)
        WoG = np.concatenate(out_rows, axis=0)  # (512, 1024)
        tables.append((np.ascontiguousarray(WfT, dtype=np.float32).astype(BF16),
                       np.ascontiguousarray(WoG, dtype=np.float32).astype(BF16)))
    return tables


# ---------------------------------------------------------------------------
# Device kernel
# ---------------------------------------------------------------------------

def build_kernel(tc, xT, wf, wo, out, reps=1, loop_iters=None, salt=""):
    import concourse.mybir as mybir
    from contextlib import ExitStack

    nc = tc.nc
    bf16 = mybir.dt.bfloat16
    f32 = mybir.dt.float32
    MULT = mybir.AluOpType.mult
    ADD = mybir.AluOpType.add

    with ExitStack() as ctx:
        consts = ctx.enter_context(tc.tile_pool(name="consts", bufs=1))
        xpool = ctx.enter_context(tc.tile_pool(name="xpool", bufs=1))
        wpool = ctx.enter_context(tc.tile_pool(name="wpool", bufs=1))
        qkvp = ctx.enter_context(tc.tile_pool(name="qkvp", bufs=3))
        kvp = ctx.enter_context(tc.tile_pool(name="kvp", bufs=3))
        scanp = ctx.enter_context(tc.tile_pool(name="scanp", bufs=2))
        up = ctx.enter_context(tc.tile_pool(name="up", bufs=3))
        tmpp = ctx.enter_context(tc.tile_pool(name="tmpp", bufs=4))
        outp = ctx.enter_context(tc.tile_pool(name="outp", bufs=4))
        psq = ctx.enter_context(tc.tile_pool(name="psq", bufs=5, space="PSUM"))
        psop = ctx.enter_context(tc.tile_pool(name="psop", bufs=3, space="PSUM"))

        # PE warmup: the Tensor engine p-state ramps to full clock only after
        # ~3us of continuous execution. Dummy matmuls during the priming-DMA
        # window ramp it up so the first real accumulation runs at 2.4GHz.
        # They rotate through the psq ring ahead of the real tiles (no reader
        # -> WAW only, no extra PSUM banks).
        wwarm = consts.tile([128, 128], bf16, name=f"wwarm{salt}")
        nc.vector.memset(wwarm[:], 0.0)
        for wi in range(20):
            pw = psq.tile([128, 128], f32, tag="psq", name=f"warm{wi}")
            nc.tensor.matmul(pw[:], wwarm[:], wwarm[:], start=True, stop=True)

        ones = consts.tile([128, SC], bf16, name=f"ones{salt}")
        nc.vector.memset(ones[:], 1.0)

        # --- DMA priming, ordered by first use ---
        # (wf_h0[mi], x_sc0[mi]) pairs feed cc0..5's full accumulation chain;
        # wf_h1 lands before the PE reaches cc6; then x_sc1, wo, x_sc2/3.
        # DRAM tensors are host-retiled so every [128,*] tile is a single
        # contiguous burst (one big descriptor) instead of 128 x 1KB strided
        # rows. x tile (sc, mi) at row (sc*NMI+mi)*128; wf tile (h, mi) at
        # row (h*NMI+mi)*128.
        def xrow(sc, mi):
            return (sc * NMI + mi) * 128

        def wfrow(h, mi):
            return (h * NMI + mi) * 128

        wf_t = [[None] * 2 for _ in range(NMI)]
        x_t = [[None] * NSC for _ in range(NMI)]
        for mi in range(NMI):
            t = wpool.tile([128, 768], bf16, tag=f"wf{mi}_0", name=f"wf{mi}_0")
            nc.sync.dma_start(out=t[:],
                              in_=wf[wfrow(0, mi):wfrow(0, mi) + 128, :])
            wf_t[mi][0] = t
            tx = xpool.tile([128, SC], bf16, tag=f"x{mi}_0", name=f"x{mi}_0")
            nc.sync.dma_start(out=tx[:], in_=xT[xrow(0, mi):xrow(0, mi) + 128, :])
            x_t[mi][0] = tx
        for mi in range(NMI):
            t = wpool.tile([128, 768], bf16, tag=f"wf{mi}_1", name=f"wf{mi}_1")
            nc.sync.dma_start(out=t[:],
                              in_=wf[wfrow(1, mi):wfrow(1, mi) + 128, :])
            wf_t[mi][1] = t
        for mi in range(NMI):
            tx = xpool.tile([128, SC], bf16, tag=f"x{mi}_1", name=f"x{mi}_1")
            nc.sync.dma_start(out=tx[:],
                              in_=xT[xrow(1, mi):xrow(1, mi) + 128, :])
            x_t[mi][1] = tx
        wo_t = []
        for ci in range(NCO):
            t = wpool.tile([128, 1024], bf16, tag=f"wo{ci}", name=f"wo{ci}")
            nc.sync.dma_start(out=t[:], in_=wo[ci * 128:(ci + 1) * 128, :])
            wo_t.append(t)
        for sc in range(2, NSC):
            for mi in range(NMI):
                tx = xpool.tile([128, SC], bf16, tag=f"x{mi}_{sc}",
                                name=f"x{mi}_{sc}")
                nc.sync.dma_start(
                    out=tx[:], in_=xT[xrow(sc, mi):xrow(sc, mi) + 128, :])
                x_t[mi][sc] = tx

        if loop_iters is not None:
            loop_cm = tc.For_i(
                0, loop_iters, 1,
                hint_engines=(mybir.EngineType.PE, mybir.EngineType.DVE,
                              mybir.EngineType.Activation, mybir.EngineType.Pool,
                              mybir.EngineType.SP))
            loop_cm.__enter__()

        def emit_qkv_vector(rep, sc, prev_scan):
            chunks = []
            for cc in range(NCC):
                ps = psq.tile([128, SC], f32, tag="psq", name=f"psq{rep}_{sc}_{cc}")
                h, cch = divmod(cc, 6)
                for mi in range(NMI):
                    nc.tensor.matmul(
                        ps[:], wf_t[mi][h][:, cch * 128:(cch + 1) * 128],
                        x_t[mi][sc][:], start=(mi == 0), stop=(mi == NMI - 1))
                sb = qkvp.tile([128, SC], bf16, tag=f"qkv{cc}", name=f"qkv{sc}_{cc}")
                nc.any.tensor_copy(sb[:], ps[:])
                chunks.append(sb)

            U = []
            for pair in range(2):
                Kre, Kim, Vre, Vim, Qre, Qim = chunks[6 * pair:6 * pair + 6]
                # lanes {0,1}: DC / Nyquist real-only fixup (adjacent via the
                # host-side channel permutation -> one 2-partition op each)
                fx = slice(0, 2)

                t1 = tmpp.tile([128, SC], bf16, tag="t1", name=f"t1_{sc}_{pair}")
                t2 = tmpp.tile([128, SC], bf16, tag="t2", name=f"t2_{sc}_{pair}")
                KVre = kvp.tile([128, SC], bf16, tag=f"kvre{pair}", name=f"kvre{sc}_{pair}")
                KVim = kvp.tile([128, SC], bf16, tag=f"kvim{pair}", name=f"kvim{sc}_{pair}")
                nc.vector.tensor_mul(t1[:], Kre[:], Vre[:])
                nc.vector.tensor_mul(t2[:], Kim[:], Vim[:])
                nc.vector.tensor_sub(KVre[:], t1[:], t2[:])
                t3 = tmpp.tile([128, SC], bf16, tag="t3", name=f"t3_{sc}_{pair}")
                t4 = tmpp.tile([128, SC], bf16, tag="t4", name=f"t4_{sc}_{pair}")
                nc.vector.tensor_mul(t3[:], Kre[:], Vim[:])
                nc.vector.tensor_mul(t4[:], Kim[:], Vre[:])
                nc.vector.tensor_add(KVim[:], t3[:], t4[:])
                nc.gpsimd.tensor_mul(KVre[fx, :], Kre[fx, :], Vre[fx, :])
                nc.gpsimd.tensor_mul(KVim[fx, :], Kim[fx, :], Vim[fx, :])

                KVre_c = scanp.tile([128, SC], f32, tag=f"scre{pair}", name=f"scre{sc}_{pair}")
                KVim_c = scanp.tile([128, SC], f32, tag=f"scim{pair}", name=f"scim{sc}_{pair}")
                init_re = 0.0 if sc == 0 else prev_scan[(pair, 0)][:, SC - 1:SC]
                init_im = 0.0 if sc == 0 else prev_scan[(pair, 1)][:, SC - 1:SC]
                nc.vector.tensor_tensor_scan(
                    KVre_c[:], ones[:], KVre[:], init_re, MULT, ADD)
                nc.vector.tensor_tensor_scan(
                    KVim_c[:], ones[:], KVim[:], init_im, MULT, ADD)
                prev_scan[(pair, 0)] = KVre_c
                prev_scan[(pair, 1)] = KVim_c

                u1 = tmpp.tile([128, SC], f32, tag="u1", name=f"u1_{sc}_{pair}")
                u2 = tmpp.tile([128, SC], f32, tag="u2", name=f"u2_{sc}_{pair}")
                Ure = up.tile([128, SC], bf16, tag=f"ure{pair}", name=f"ure{sc}_{pair}")
                Uim = up.tile([128, SC], bf16, tag=f"uim{pair}", name=f"uim{sc}_{pair}")
                nc.vector.tensor_mul(u1[:], KVre_c[:], Qre[:])
                nc.vector.tensor_mul(u2[:], KVim_c[:], Qim[:])
                nc.vector.tensor_add(Ure[:], u1[:], u2[:])
                u3 = tmpp.tile([128, SC], f32, tag="u3", name=f"u3_{sc}_{pair}")
                u4 = tmpp.tile([128, SC], f32, tag="u4", name=f"u4_{sc}_{pair}")
                nc.vector.tensor_mul(u3[:], KVim_c[:], Qre[:])
                nc.vector.tensor_mul(u4[:], KVre_c[:], Qim[:])
                nc.vector.tensor_sub(Uim[:], u3[:], u4[:])
                nc.gpsimd.tensor_mul(Ure[fx, :], KVre_c[fx, :], Qre[fx, :])
                nc.gpsimd.tensor_mul(Uim[fx, :], KVim_c[fx, :], Qim[fx, :])
                U += [Ure, Uim]
            return U

        def emit_out(sc, U):
            for mo in range(8):
                po = psop.tile([128, SC], f32, tag="pso", name=f"pso{sc}_{mo}")
                for ci in range(NCO):
                    nc.tensor.matmul(
                        po[:], wo_t[ci][:, mo * 128:(mo + 1) * 128], U[ci][:],
                        start=(ci == 0), stop=(ci == NCO - 1))
                so = outp.tile([128, SC], bf16, tag="so", name=f"so{sc}_{mo}")
                nc.any.tensor_copy(so[:], po[:])
                orow = (sc * 8 + mo) * 128
                nc.sync.dma_start(out=out[orow:orow + 128, :], in_=so[:])

        # Software pipelining: emit s-chunk sc+1's qkv matmuls BEFORE s-chunk
        # sc's output matmuls so the PE keeps streaming qkv work while the
        # DVE bind/scan/unbind chain for the previous chunk produces U.
        for rep in range(reps):
            prev_scan = {}
            pend = None
            for sc in range(NSC):
                U = emit_qkv_vector(rep, sc, prev_scan)
                if pend is not None:
                    emit_out(*pend)
                pend = (sc, U)
            emit_out(*pend)
        if loop_iters is not None:
            loop_cm.__exit__(None, None, None)


def build_bass(reps=1, loop_iters=None, salt=""):
    import concourse.bacc as bacc
    import concourse.tile as tile
    import concourse.mybir as mybir

    nc = bacc.Bacc("TRN2", target_bir_lowering=False, debug=False, num_devices=8)
    # tiled layouts: x as NSC*NMI contiguous [128,512] tiles, wf as 2*NMI
    # contiguous [128,768] tiles, out as NSC*8 contiguous [128,512] tiles
    xT = nc.dram_tensor("xT", [4 * M, SC], mybir.dt.bfloat16, kind="ExternalInput")
    wf = nc.dram_tensor("wf", [2 * M, 768], mybir.dt.bfloat16, kind="ExternalInput")
    wo = nc.dram_tensor("wo", [512, M], mybir.dt.bfloat16, kind="ExternalInput")
    out = nc.dram_tensor("out", [4 * M, SC], mybir.dt.bfloat16,
                         kind="ExternalOutput")
    with tile.TileContext(nc) as tc:
        build_kernel(tc, xT[:], wf[:], wo[:], out[:], reps=reps,
                     loop_iters=loop_iters, salt=salt)
    nc.compile()
    return nc


_NC_CACHE = {}


def _get_nc(reps=1, loop_iters=None, salt=""):
    key = (reps, loop_iters, salt)
    if key not in _NC_CACHE:
        _NC_CACHE[key] = build_bass(reps, loop_iters, salt)
    return _NC_CACHE[key]


def make_in_maps(x, W_qkv, W_o):
    tables = build_tables(W_qkv, W_o)
    x = np.asarray(x, dtype=np.float32)
    in_maps = []
    for core in range(8):
        b = core // 2
        xT_c = np.ascontiguousarray(x[b].T).astype(BF16)  # [M, S]
        # tile (sc, mi): [NSC, NMI, 128, SC] -> rows (sc*NMI+mi)*128
        xt = np.ascontiguousarray(
            xT_c.reshape(NMI, 128, NSC, SC).transpose(2, 0, 1, 3)
        ).reshape(4 * M, SC)
        WfT, WoG = tables[core]
        # tile (h, mi): [2, NMI, 128, 768] -> rows (h*NMI+mi)*128
        wft = np.ascontiguousarray(
            WfT.reshape(NMI, 128, 2, 768).transpose(2, 0, 1, 3)
        ).reshape(2 * M, 768)
        in_maps.append({"xT": xt, "wf": wft, "wo": WoG})
    return in_maps


def combine_outputs(results):
    out = np.empty((B, S, M), dtype=np.float32)
    for b in range(B):
        acc = results[2 * b]["out"].astype(np.float32) + \
            results[2 * b + 1]["out"].astype(np.float32)
        # tiled [NSC*8*128, SC]: tile (sc, mo) -> full[mo*128+p, sc*SC+c]
        full = acc.reshape(NSC, 8, 128, SC).transpose(1, 2, 0, 3).reshape(M, S)
        out[b] = full.T
    return out


def kernel(x, W_qkv, W_o):
    from concourse.bass_utils import run_bass_kernel_spmd
    nc = _get_nc()
    in_maps = make_in_maps(x, W_qkv, W_o)
    res = run_bass_kernel_spmd(nc, in_maps, core_ids=list(range(8)))
    return combine_outputs(res.results)
